# revision 5
# baseline (speedup 1.0000x reference)
"""Trainium2 Bass kernel: AdditiveAttention-style scoring head.

Computes, for x:(B,N,D), W1/W2:(A,D), b1/b2:(A,), Wout:(A,), bout:(1,):
    x1 = x @ W1.T + b1                       (B,N,A)
    x2 = x @ W2.T + b2                       (B,N,A)
    out[b,i-1,j] = sum_a Wout[a]*tanh(x1[b,j,a] + x2[b,i,a]) + bout,  i=1..N-1

Sharding: data-parallel over batch B across 8 NeuronCores (B/8=4 per core),
weights replicated, no collectives. Per core the 33M-element tanh stream is
the roofline (ACT engine, 128 lanes @1.2GHz); the broadcast add runs on DVE
(bf16 2x mode via duplicated-pair APs), and the A-reduction against Wout runs
on the TensorEngine as accumulating K=128 matmuls (M=1), with bout folded in
via a K=1 init-matmul against a ones row.
"""
import sys
import numpy as np

if "/opt/trn_rl_repo" not in sys.path:
    sys.path.insert(0, "/opt/trn_rl_repo")

B, N, D, A = 32, 128, 512, 512
NCORES = 8
BPC = B // NCORES      # batches per core
KC = D // 128          # contraction chunks for the input matmuls
MC = A // 128          # a-chunks (partition dim of the fused stage)
IB = 32                # i-rows per pipeline block
NIB = N // IB          # i-blocks per batch (covers i=0..N-1; i=0 dropped at DMA)
F = IB * N             # free elements per (b, iblock) tile
REG = 2048             # psum output region (f32 elems) = 4 banks
RPI = F // REG         # regions per iblock
MMN = 512              # matmul free dim (one psum bank)

_CACHE = {}


def _build_nc():
    import concourse.bass as bass
    import concourse.bacc as bacc
    import concourse.mybir as mybir
    from concourse import tile

    f32 = mybir.dt.float32
    bf16 = mybir.dt.bfloat16
    AF = mybir.ActivationFunctionType

    nc = bacc.Bacc(None, target_bir_lowering=False)

    xT = nc.declare_dram_parameter("xT", [D, BPC * N], f32, isOutput=False)
    w1t = nc.declare_dram_parameter("w1t", [D, A], f32, isOutput=False)
    w2t = nc.declare_dram_parameter("w2t", [D, A], f32, isOutput=False)
    b1c = nc.declare_dram_parameter("b1c", [128, MC], f32, isOutput=False)
    b2c = nc.declare_dram_parameter("b2c", [128, MC], f32, isOutput=False)
    woutc = nc.declare_dram_parameter("woutc", [128, MC], f32, isOutput=False)
    boutp = nc.declare_dram_parameter("bout", [1, 1], f32, isOutput=False)
    out = nc.declare_dram_parameter("out", [BPC, (N - 1) * N], f32, isOutput=True)

    with tile.TileContext(nc) as tc:
        with (
            tc.tile_pool(name="const", bufs=1) as cpool,
            tc.tile_pool(name="xw", bufs=1) as xwpool,
            tc.tile_pool(name="x12", bufs=1) as xpool,
            tc.tile_pool(name="s", bufs=3) as spool,
            tc.tile_pool(name="t", bufs=8) as tpool,
            tc.tile_pool(name="stage", bufs=4) as stpool,
        ):
            # ---- input loads ----
            xT_sb, w1_sb, w2_sb = [], [], []
            for k in range(KC):
                tx = xwpool.tile([128, BPC * N], f32, tag=f"xT{k}")
                nc.sync.dma_start(tx[:, :], xT[k * 128:(k + 1) * 128, :])
                xT_sb.append(tx)
                t1 = xwpool.tile([128, A], f32, tag=f"w1{k}")
                nc.sync.dma_start(t1[:, :], w1t[k * 128:(k + 1) * 128, :])
                w1_sb.append(t1)
                t2 = xwpool.tile([128, A], f32, tag=f"w2{k}")
                nc.sync.dma_start(t2[:, :], w2t[k * 128:(k + 1) * 128, :])
                w2_sb.append(t2)
            b1_sb = cpool.tile([128, MC], f32, tag="b1")
            nc.sync.dma_start(b1_sb[:, :], b1c[:, :])
            b2_sb = cpool.tile([128, MC], f32, tag="b2")
            nc.sync.dma_start(b2_sb[:, :], b2c[:, :])
            woutf = cpool.tile([128, MC], f32, tag="woutf")
            nc.sync.dma_start(woutf[:, :], woutc[:, :])
            wout_sb = cpool.tile([128, MC], bf16, tag="wout")
            nc.vector.tensor_copy(wout_sb[:, :], woutf[:, :])
            boutf = cpool.tile([1, 1], f32, tag="boutf")
            nc.sync.dma_start(boutf[:, :], boutp[:, :])

            x1_sb = [xpool.tile([128, BPC * N], bf16, tag=f"x1_{c}", name=f"x1_{c}") for c in range(MC)]
            x2_sb = [xpool.tile([128, BPC * N], bf16, tag=f"x2_{c}", name=f"x2_{c}") for c in range(MC)]
            x2d_sb = [xpool.tile([128, BPC * N * 2], bf16, tag=f"x2d_{c}", name=f"x2d_{c}") for c in range(MC)]

            # ---- x1/x2 = W @ x^T + b, in [a_chunk, (b,n)] layout, cast bf16 ----
            with tc.tile_pool(name="psA", bufs=2, space=bass.MemorySpace.PSUM) as psA:
                for w_sb, bvec, dst in ((w1_sb, b1_sb, x1_sb), (w2_sb, b2_sb, x2_sb)):
                    for m in range(MC):
                        ps = psA.tile([128, BPC * N], f32, tag="psA")
                        for k in range(KC):
                            nc.tensor.matmul(
                                ps[:, :],
                                w_sb[k][:, m * 128:(m + 1) * 128],
                                xT_sb[k][:, :],
                                start=(k == 0),
                                stop=(k == KC - 1),
                            )
                        nc.vector.tensor_scalar_add(dst[m][:, :], ps[:, :], bvec[:, m:m + 1])

            # x2 duplicated-pairs copy: x2d[:, 2q+t] = x2[:, q]  (t in {0,1})
            for c in range(MC):
                src = x2_sb[c][:, :]
                dst = x2d_sb[c][:, :]
                in_ap = bass.AP(src.tensor, src.offset,
                                [[src.ap[0][0], 128], [1, BPC * N], [0, 2]])
                out_ap = bass.AP(dst.tensor, dst.offset,
                                 [[dst.ap[0][0], 128], [2, BPC * N], [1, 2]])
                nc.vector.tensor_copy(out_ap, in_ap)

            # ---- main pipeline: DVE add -> ACT tanh -> PE reduce -> DMA out ----
            with tc.tile_pool(name="psO", bufs=2, space=bass.MemorySpace.PSUM) as psO:
                for b in range(BPC):
                    for ib in range(NIB):
                        t_tiles = []
                        for c in range(MC):
                            s = spool.tile([128, F], bf16, tag="s")
                            sap = s[:, :]
                            x1ap = x1_sb[c][:, b * N:(b + 1) * N]
                            in0 = bass.AP(x1ap.tensor, x1ap.offset,
                                          [[x1ap.ap[0][0], 128], [0, IB], [2, N // 2], [1, 2]])
                            x2ap = x2d_sb[c][:, :]
                            in1 = bass.AP(x2ap.tensor, x2ap.offset + (b * N + ib * IB) * 2,
                                          [[x2ap.ap[0][0], 128], [2, IB], [0, N // 2], [1, 2]])
                            sout = bass.AP(sap.tensor, sap.offset,
                                           [[sap.ap[0][0], 128], [N, IB], [2, N // 2], [1, 2]])
                            nc.vector.tensor_tensor(sout, in0, in1, mybir.AluOpType.add)
                            tt = tpool.tile([128, F], bf16, tag="t")
                            nc.scalar.activation(tt[:, :], s[:, :], AF.Tanh)
                            t_tiles.append(tt)
                        for r in range(RPI):
                            ps = psO.tile([1, REG], f32, tag="psO")
                            for sl in range(REG // MMN):
                                dstp = ps[0:1, sl * MMN:(sl + 1) * MMN]
                                col0 = r * REG + sl * MMN
                                for c in range(MC):
                                    nc.tensor.matmul(dstp, wout_sb[:, c:c + 1],
                                                     t_tiles[c][:, col0:col0 + MMN],
                                                     start=(c == 0), stop=(c == MC - 1))
                            # stage PSUM->SBUF with +bout fused, then DMA out
                            i0 = ib * IB + r * (REG // N)
                            if i0 == 0:
                                stg = stpool.tile([1, REG - N], f32, tag="stg0")
                                nc.vector.tensor_scalar_add(stg[:, :], ps[0:1, N:REG],
                                                            boutf[0:1, 0:1])
                                nc.sync.dma_start(out[b:b + 1, 0:REG - N], stg[:, :])
                            else:
                                stg = stpool.tile([1, REG], f32, tag="stg")
                                nc.vector.tensor_scalar_add(stg[:, :], ps[0:1, :],
                                                            boutf[0:1, 0:1])
                                o0 = (i0 - 1) * N
                                nc.sync.dma_start(out[b:b + 1, o0:o0 + REG], stg[:, :])

    nc.finalize()
    return nc


def _get_nc():
    if "nc" not in _CACHE:
        _CACHE["nc"] = _build_nc()
    return _CACHE["nc"]


def _prep_in_maps(x, W1, b1, W2, b2, Wout, bout):
    f = np.float32
    w1t = np.ascontiguousarray(np.asarray(W1, f).T)
    w2t = np.ascontiguousarray(np.asarray(W2, f).T)
    b1v = np.ascontiguousarray(np.asarray(b1, f).reshape(MC, 128).T)
    b2v = np.ascontiguousarray(np.asarray(b2, f).reshape(MC, 128).T)
    wov = np.ascontiguousarray(np.asarray(Wout, f).reshape(MC, 128).T)
    bov = np.asarray(bout, f).reshape(1, 1)
    x = np.asarray(x, f)
    in_maps = []
    for ci in range(NCORES):
        xs = x[ci * BPC:(ci + 1) * BPC]
        xTi = np.ascontiguousarray(xs.transpose(2, 0, 1).reshape(D, BPC * N))
        in_maps.append({
            "xT": xTi, "w1t": w1t, "w2t": w2t,
            "b1c": b1v, "b2c": b2v, "woutc": wov, "bout": bov,
        })
    return in_maps


def _run(x, W1, b1, W2, b2, Wout, bout, trace=False):
    from concourse.bass_utils import run_bass_kernel_spmd

    nc = _get_nc()
    in_maps = _prep_in_maps(x, W1, b1, W2, b2, Wout, bout)
    res = run_bass_kernel_spmd(nc, in_maps, core_ids=list(range(NCORES)), trace=trace)
    outs = [np.asarray(res.results[ci]["out"]).reshape(BPC, N - 1, N)
            for ci in range(NCORES)]
    full = np.concatenate(outs, axis=0).astype(np.float32)
    return full, res


def kernel(x, W1, b1, W2, b2, Wout, bout):
    full, _ = _run(x, W1, b1, W2, b2, Wout, bout, trace=False)
    return full
